# revision 20
# baseline (speedup 1.0000x reference)
"""LoRA embedding lookup on 8 Trainium2 NeuronCores.

out[b, s, :] = weight[ids[b, s], :] + SCALING * (lora_B[ids[b, s], :] @ lora_A)

Sharding: tokens are split across the 8 cores (batch row c -> core c).
Each core holds the full tables in its HBM, gathers its 2048 rows with
indirect DMA, runs the rank-16 delta matmul on the PE, adds, and writes
a disjoint slice of the output. No collectives needed.

Device-side layout tricks:
- weight (fp16) and lora_B (bf16 bits packed in fp16 slots) are fused
  host-side into one [VOCAB, 1040] fp16 table, so a single indirect-DMA
  descriptor per token fetches both the embedding row and its LoRA-B row.
- The rank-16 delta matmul runs in bf16 on the PE with f32 PSUM accumulate.
- The output is written fp16 and upcast to f32 on the host.
Measured accuracy vs the f32 reference: max abs err 8.7e-5 on an output
scale of 0.11 (the bf16 LoRA product dominates; fp16 weight/output
rounding adds <20%).
"""

import numpy as np
import ml_dtypes

try:
    import concourse.bass as bass
except ImportError:  # fresh grading dir without the default PYTHONPATH
    import sys

    sys.path.insert(0, "/opt/trn_rl_repo")
    import concourse.bass as bass

import concourse.mybir as mybir
import concourse.tile as tile
from concourse import bacc
from concourse.bass_utils import run_bass_kernel_spmd

VOCAB = 50257
DIM = 1024
RANK = 16
ROW = DIM + RANK  # fused table row (fp16 elements): [w fp16 | b bf16-bits]
SCALING = 32.0 / 16.0  # alpha / rank
N_CORES = 8
TOK_PER_CORE = 2048
P = 128
N_TILES = TOK_PER_CORE // P  # 16
NSPLIT = 2  # PSUM bank limit: matmul N <= 512
STORE_GROUP = 2  # token tiles per output store

_cached_nc = None


def _build_nc():
    global _cached_nc
    if _cached_nc is not None:
        return _cached_nc

    f32 = mybir.dt.float32
    f16 = mybir.dt.float16
    bf16 = mybir.dt.bfloat16

    nc = bacc.Bacc(None, target_bir_lowering=False, dynamic_dma_scratch_size=65536)
    ids_d = nc.declare_dram_parameter("ids", [P, N_TILES], mybir.dt.int32, isOutput=False)
    t_d = nc.declare_dram_parameter("table", [VOCAB, ROW], f16, isOutput=False)
    a_d = nc.declare_dram_parameter("lora_a", [RANK, DIM], f32, isOutput=False)
    out_d = nc.declare_dram_parameter("out", [TOK_PER_CORE, DIM], f16, isOutput=True)

    with tile.TileContext(nc) as tc:
        with (
            tc.tile_pool(name="const", bufs=1) as const_tp,
            tc.tile_pool(name="cp", bufs=12) as cp,
            tc.tile_pool(name="btp", bufs=8) as btp,
            tc.tile_pool(name="op", bufs=6) as op,
            tc.tile_pool(name="pst", bufs=1, space="PSUM") as pst,
            tc.tile_pool(name="psw", bufs=1, space="PSUM") as psw,
            tc.tile_pool(name="psd", bufs=3, space="PSUM") as psd,
        ):
            from concourse.masks import make_identity

            ids_sb = const_tp.tile([P, N_TILES], mybir.dt.int32)
            nc.sync.dma_start(out=ids_sb[:], in_=ids_d[:])

            identity = const_tp.tile([P, P], bf16)
            make_identity(nc, identity[:])

            a_sb = const_tp.tile([RANK, DIM], f32)
            nc.sync.dma_start(out=a_sb[:], in_=a_d[:])
            a_bf = const_tp.tile([RANK, DIM], bf16)
            nc.vector.tensor_scalar_mul(a_bf[:], a_sb[:], SCALING)

            # ~4.5us of dummy back-to-back matmuls while the first gathers
            # are in flight: triggers the PE's HAM 2.4GHz clock (it idles at
            # 1.2GHz otherwise, and the real matmul stream is too sparse to
            # warm it). Results are never read; the tile shares the d_ps tag.
            wr = const_tp.tile([P, 512], bf16)
            nc.vector.memset(wr[:], 0.0)
            warm = psd.tile([P, DIM], f32, tag="d_ps")
            for _ in range(11):
                nc.tensor.matmul(warm[:, :512], identity[:], wr[:], start=True, stop=True)
            # Re-warm scratch for one dummy matmul per tile (dedicated PSUM
            # bank, so it never contends with the real pipeline's slots).
            warm2 = psw.tile([RANK, 512], f32)

            for g in range(N_TILES // STORE_GROUP):
                out_big = op.tile([P, STORE_GROUP * DIM], f16)
                for k in range(STORE_GROUP):
                    j = g * STORE_GROUP + k
                    # Gather 128 fused rows (one per partition) for this tile.
                    c_tile = cp.tile([P, ROW], f16)
                    nc.gpsimd.indirect_dma_start(
                        out=c_tile[:],
                        out_offset=None,
                        in_=t_d[:],
                        in_offset=bass.IndirectOffsetOnAxis(
                            ap=ids_sb[:, j : j + 1], axis=0
                        ),
                    )
                    w_ap = c_tile[:, :DIM]
                    b_bf = c_tile[:, DIM:ROW].bitcast(bf16)

                    # Keep the PE's HAM clock warm mid-stream: one dummy
                    # matmul per tile, gated on this tile's gather.
                    nc.tensor.matmul(
                        warm2[:], c_tile[:, :RANK], c_tile[:, :512],
                        start=True, stop=True,
                    )

                    # bT = b.T : [RANK, P] so tokens land on PSUM partitions.
                    bT_ps = pst.tile([RANK, P], bf16)
                    nc.tensor.transpose(out=bT_ps[:], in_=b_bf, identity=identity[:])
                    bT = btp.tile([RANK, P], bf16)
                    nc.scalar.copy(out=bT[:], in_=bT_ps[:])

                    # delta = b @ (SCALING * lora_A) : [P, DIM], f32 accumulate
                    d_ps = psd.tile([P, DIM], f32)
                    for h in range(NSPLIT):
                        sl = slice(h * (DIM // NSPLIT), (h + 1) * (DIM // NSPLIT))
                        nc.tensor.matmul(
                            d_ps[:, sl], bT[:], a_bf[:, sl], start=True, stop=True
                        )

                    nc.vector.tensor_add(
                        out=out_big[:, k * DIM : (k + 1) * DIM],
                        in0=w_ap,
                        in1=d_ps[:],
                    )
                # One store for STORE_GROUP tiles; DRAM rows (g*SG+k)*128+p
                # live at [p, k, :] of the rearranged view.
                dest = out_d[
                    g * STORE_GROUP * P : (g + 1) * STORE_GROUP * P, :
                ].rearrange("(k p) d -> p k d", k=STORE_GROUP)
                nc.sync.dma_start(out=dest, in_=out_big[:])

    nc.compile()
    _cached_nc = nc
    return nc


def run(inputs, **spmd_kwargs):
    """Run on 8 cores; returns (full_output, BassKernelResults)."""
    ids = np.ascontiguousarray(np.asarray(inputs["input_ids"]).astype(np.int32)).reshape(-1)
    weight = np.asarray(inputs["weight"], dtype=np.float32)
    lora_a = np.ascontiguousarray(np.asarray(inputs["lora_A"], dtype=np.float32))
    lora_b = np.asarray(inputs["lora_B"], dtype=np.float32)
    assert ids.shape == (N_CORES * TOK_PER_CORE,)
    assert weight.shape == (VOCAB, DIM)
    assert lora_a.shape == (RANK, DIM)
    assert lora_b.shape == (VOCAB, RANK)

    table = np.empty((VOCAB, ROW), dtype=np.float16)
    table[:, :DIM] = weight.astype(np.float16)
    table[:, DIM:] = lora_b.astype(ml_dtypes.bfloat16).view(np.float16)

    nc = _build_nc()
    in_maps = []
    for c in range(N_CORES):
        chunk = ids[c * TOK_PER_CORE : (c + 1) * TOK_PER_CORE]
        # ids_dev[p, j] = chunk[j * P + p] -> tile j gathers tokens j*P .. j*P+127
        ids_dev = np.ascontiguousarray(chunk.reshape(N_TILES, P).T)
        in_maps.append({"ids": ids_dev, "table": table, "lora_a": lora_a})
    res = run_bass_kernel_spmd(nc, in_maps, list(range(N_CORES)), **spmd_kwargs)
    out = np.stack([res.results[c]["out"] for c in range(N_CORES)], axis=0)
    return out.astype(np.float32), res


def kernel(**inputs):
    out, _ = run(inputs)
    return out


# revision 21
# speedup vs baseline: 1.0731x; 1.0731x over previous
"""LoRA embedding lookup on 8 Trainium2 NeuronCores.

out[b, s, :] = weight[ids[b, s], :] + SCALING * (lora_B[ids[b, s], :] @ lora_A)

Sharding: tokens are split across the 8 cores (batch row c -> core c).
Each core holds the full tables in its HBM, gathers its 2048 rows with
indirect DMA, runs the rank-16 delta matmul on the PE, adds, and writes
a disjoint slice of the output. No collectives needed.

Device-side layout tricks:
- weight (fp16) and lora_B (bf16 bits packed in fp16 slots) are fused
  host-side into one [VOCAB, 1040] fp16 table, so a single indirect-DMA
  descriptor per token fetches both the embedding row and its LoRA-B row.
- The rank-16 delta matmul runs in bf16 on the PE with f32 PSUM accumulate.
- The output is written fp16 and upcast to f32 on the host.
Measured accuracy vs the f32 reference: max abs err 8.7e-5 on an output
scale of 0.11 (the bf16 LoRA product dominates; fp16 weight/output
rounding adds <20%).
"""

import numpy as np
import ml_dtypes

try:
    import concourse.bass as bass
except ImportError:  # fresh grading dir without the default PYTHONPATH
    import sys

    sys.path.insert(0, "/opt/trn_rl_repo")
    import concourse.bass as bass

import concourse.mybir as mybir
import concourse.tile as tile
from concourse import bacc
from concourse.bass_utils import run_bass_kernel_spmd

VOCAB = 50257
DIM = 1024
RANK = 16
ROW = DIM + RANK  # fused table row (fp16 elements): [w fp16 | b bf16-bits]
SCALING = 32.0 / 16.0  # alpha / rank
N_CORES = 8
TOK_PER_CORE = 2048
P = 128
N_TILES = TOK_PER_CORE // P  # 16
NSPLIT = 2  # PSUM bank limit: matmul N <= 512
STORE_GROUP = 2  # token tiles per output store

_cached_nc = None


def _build_nc():
    global _cached_nc
    if _cached_nc is not None:
        return _cached_nc

    f32 = mybir.dt.float32
    f16 = mybir.dt.float16
    bf16 = mybir.dt.bfloat16

    nc = bacc.Bacc(None, target_bir_lowering=False, dynamic_dma_scratch_size=65536)
    ids_d = nc.declare_dram_parameter("ids", [P, N_TILES], mybir.dt.int32, isOutput=False)
    t_d = nc.declare_dram_parameter("table", [VOCAB, ROW], f16, isOutput=False)
    a_d = nc.declare_dram_parameter("lora_a", [RANK, DIM], f32, isOutput=False)
    out_d = nc.declare_dram_parameter("out", [TOK_PER_CORE, DIM], f16, isOutput=True)

    with tile.TileContext(nc) as tc:
        with (
            tc.tile_pool(name="const", bufs=1) as const_tp,
            tc.tile_pool(name="cp", bufs=12) as cp,
            tc.tile_pool(name="btp", bufs=8) as btp,
            tc.tile_pool(name="op", bufs=6) as op,
            tc.tile_pool(name="pst", bufs=2, space="PSUM") as pst,
            tc.tile_pool(name="psd", bufs=3, space="PSUM") as psd,
        ):
            from concourse.masks import make_identity

            ids_sb = const_tp.tile([P, N_TILES], mybir.dt.int32)
            nc.sync.dma_start(out=ids_sb[:], in_=ids_d[:])

            identity = const_tp.tile([P, P], bf16)
            make_identity(nc, identity[:])

            a_sb = const_tp.tile([RANK, DIM], f32)
            nc.sync.dma_start(out=a_sb[:], in_=a_d[:])
            a_bf = const_tp.tile([RANK, DIM], bf16)
            nc.vector.tensor_scalar_mul(a_bf[:], a_sb[:], SCALING)

            # ~4.5us of dummy back-to-back matmuls while the first gathers
            # are in flight: triggers the PE's HAM 2.4GHz clock (it idles at
            # 1.2GHz otherwise, and the real matmul stream is too sparse to
            # warm it). Results are never read; the tile shares the d_ps tag.
            wr = const_tp.tile([P, 512], bf16)
            nc.vector.memset(wr[:], 0.0)
            warm = psd.tile([P, DIM], f32, tag="d_ps")
            for _ in range(11):
                nc.tensor.matmul(warm[:, :512], identity[:], wr[:], start=True, stop=True)

            for g in range(N_TILES // STORE_GROUP):
                out_big = op.tile([P, STORE_GROUP * DIM], f16)
                for k in range(STORE_GROUP):
                    j = g * STORE_GROUP + k
                    # Gather 128 fused rows (one per partition) for this tile.
                    c_tile = cp.tile([P, ROW], f16)
                    nc.gpsimd.indirect_dma_start(
                        out=c_tile[:],
                        out_offset=None,
                        in_=t_d[:],
                        in_offset=bass.IndirectOffsetOnAxis(
                            ap=ids_sb[:, j : j + 1], axis=0
                        ),
                    )
                    w_ap = c_tile[:, :DIM]
                    b_bf = c_tile[:, DIM:ROW].bitcast(bf16)

                    # bT = b.T : [RANK, P] so tokens land on PSUM partitions.
                    bT_ps = pst.tile([RANK, P], bf16)
                    nc.tensor.transpose(out=bT_ps[:], in_=b_bf, identity=identity[:])
                    bT = btp.tile([RANK, P], bf16)
                    nc.scalar.copy(out=bT[:], in_=bT_ps[:])

                    # delta = b @ (SCALING * lora_A) : [P, DIM], f32 accumulate
                    d_ps = psd.tile([P, DIM], f32)
                    for h in range(NSPLIT):
                        sl = slice(h * (DIM // NSPLIT), (h + 1) * (DIM // NSPLIT))
                        nc.tensor.matmul(
                            d_ps[:, sl], bT[:], a_bf[:, sl], start=True, stop=True
                        )

                    nc.vector.tensor_add(
                        out=out_big[:, k * DIM : (k + 1) * DIM],
                        in0=w_ap,
                        in1=d_ps[:],
                    )
                # One store for STORE_GROUP tiles; DRAM rows (g*SG+k)*128+p
                # live at [p, k, :] of the rearranged view.
                dest = out_d[
                    g * STORE_GROUP * P : (g + 1) * STORE_GROUP * P, :
                ].rearrange("(k p) d -> p k d", k=STORE_GROUP)
                nc.sync.dma_start(out=dest, in_=out_big[:])

    nc.compile()
    _cached_nc = nc
    return nc


def run(inputs, **spmd_kwargs):
    """Run on 8 cores; returns (full_output, BassKernelResults)."""
    ids = np.ascontiguousarray(np.asarray(inputs["input_ids"]).astype(np.int32)).reshape(-1)
    weight = np.asarray(inputs["weight"], dtype=np.float32)
    lora_a = np.ascontiguousarray(np.asarray(inputs["lora_A"], dtype=np.float32))
    lora_b = np.asarray(inputs["lora_B"], dtype=np.float32)
    assert ids.shape == (N_CORES * TOK_PER_CORE,)
    assert weight.shape == (VOCAB, DIM)
    assert lora_a.shape == (RANK, DIM)
    assert lora_b.shape == (VOCAB, RANK)

    table = np.empty((VOCAB, ROW), dtype=np.float16)
    table[:, :DIM] = weight.astype(np.float16)
    table[:, DIM:] = lora_b.astype(ml_dtypes.bfloat16).view(np.float16)

    nc = _build_nc()
    in_maps = []
    for c in range(N_CORES):
        chunk = ids[c * TOK_PER_CORE : (c + 1) * TOK_PER_CORE]
        # ids_dev[p, j] = chunk[j * P + p] -> tile j gathers tokens j*P .. j*P+127
        ids_dev = np.ascontiguousarray(chunk.reshape(N_TILES, P).T)
        in_maps.append({"ids": ids_dev, "table": table, "lora_a": lora_a})
    res = run_bass_kernel_spmd(nc, in_maps, list(range(N_CORES)), **spmd_kwargs)
    out = np.stack([res.results[c]["out"] for c in range(N_CORES)], axis=0)
    return out.astype(np.float32), res


def kernel(**inputs):
    out, _ = run(inputs)
    return out
